# revision 13
# baseline (speedup 1.0000x reference)
"""Trainium2 Bass kernel for nn_CAM_62852551409742 (low-rank tanh rewrite, v2).

Math (reference):
  f = feats[:, :, 0, :]                               [R,B,T], R=4, B=512, T=150
  v = feats.reshape(B, K)                             [B,K], K=600
  att[r,b,t,k] = tanh(u[r,b,t] * v[b,k]),  u = a[r]*f
  Hm = relu(att @ Wc[r].T + f*W[r])                   [R,B,T,32]
  attf = Hm @ Wh[r] + f
  out = (attf-cat @ W1.T + b1) @ W2.T + b2            [B,1,7]

Key rewrite (kept from v1): tanh(uv) ~= sum_j c_j (uv)^p_j (odd powers
p=1,3,5,7,9), so att @ Wc.T = sum_j u^p_j * S_j with
S_j[r,b,c] = sum_k c_j v[b,k]^p_j Wc[r,c,k]; the f*W term folds in via an
indicator row (row 88 of the kt=4 contraction tile).

v2 pipeline (v1 profile: ~12us stall -- phi lived on 5 SBUF partitions
~= 2 of 16 SDMA ports; 28 dma_starts at ~0.7us sequencer each; 256
stage-A MMs; cold PE for the back half):

  stage D: S_T[(jfi,b), (rc)] = vpn.T @ wct as 3 M-chunks x 5 kt
           accumulating MMs (15 total; S comes off the PE already
           transposed -- no PE-transposes, no identity matrix).
  realign: 3 DVE copies -> st_sb, then 5 single-dest-partition DMAs
           -> s_all[jf, 128 b + (rc)]  (DMA partition steps are only
           legal on dim0 of BOTH access patterns, so a partition-axis
           exchange needs a single-partition side).
  expand:  4 DMAs (one per r, both sides partition-step on dim0)
           -> s_blk[5r + jf, 128 b + 32 r + c], block-diagonal over r
           with zeros from one early DVE memset.
  stage A: ONE plain matmul per batch: K=20 (5r+jf rows), M=128, N=150.
           64 MMs (was 256).  phi20 [20, 9600] (partition 5r+jf) is a
           single ~2.9us DMA over 5 SDMA engines (was ~7us over 2).
  relu:    PSUM->SBUF bf16 drain per 3 batches, DVE/ACT alternating
           (the ~6us floor: 1.23M fp32 PSUM reads at 1x).
  final:   per-t MMs vs U[(rc),t,i]=Wh*Wx in 4 col groups + f@Wx tail
           (unchanged from v1).

DMA budget (each dma_start ~0.7us of its ring's sequencer): sync ring:
vpn, realign x5, expand r0/r1, out; scalar ring: wct, phi, ft, expand
r2/r3; gpsimd (SWDGE): u, wx, bx.
"""

import os
from contextlib import ExitStack

import numpy as np
import ml_dtypes

import concourse.bacc as bacc
import concourse.bass as bass
import concourse.tile as tile
from concourse import mybir
from concourse import bass_utils
from concourse.ap import AP

R, B, T, H = 4, 512, 150, 32
K = R * T                      # 600
NCORES = 8
BL = B // NCORES               # 64 batches per core
JF = 5                         # odd powers 1..9
POWS = (1, 3, 5, 7, 9)
KTS = [(0, 128), (128, 128), (256, 128), (384, 128), (512, 88)]
KPD = [128, 128, 128, 128, 89]   # stage-D contraction (kt4 + indicator row)
F32 = mybir.dt.float32
BF16 = mybir.dt.bfloat16
BF = ml_dtypes.bfloat16

_CACHE = {}


def build_nc():
    nc = bacc.Bacc("TRN2", target_bir_lowering=False)
    # vpn cols: (jf, b) -> 64*jf + b
    vpn_d = nc.dram_tensor("vpn", [128, 5, 320], BF16, kind="ExternalInput")
    wct_d = nc.dram_tensor("wct", [128, 5, 128], BF16, kind="ExternalInput")
    # phi[32r + jf, 150*b + t] = u[r, b0+b, t]^p_jf (per-strip chunks)
    phi_d = nc.dram_tensor("phi", [4, 5, BL * T], BF16, kind="ExternalInput")
    u_d = nc.dram_tensor("u", [128, T, 7], BF16, kind="ExternalInput")
    ft_d = nc.dram_tensor("ft", [128, 5, BL], BF16, kind="ExternalInput")
    wx_d = nc.dram_tensor("wx", [128, 5, 7], BF16, kind="ExternalInput")
    bx_d = nc.dram_tensor("bx", [7, 1], F32, kind="ExternalInput")
    out_d = nc.dram_tensor("out", [7, BL], F32, kind="ExternalOutput")
    dbg = os.environ.get("KDEBUG")
    if dbg:
        dsb_d = nc.dram_tensor("dsb", [128, 8192], BF16, kind="ExternalOutput")

    with tile.TileContext(nc) as tc, ExitStack() as ctx:
        consts = ctx.enter_context(tc.tile_pool(name="consts", bufs=1))
        hmp = ctx.enter_context(tc.tile_pool(name="hm", bufs=1))
        ps_d = ctx.enter_context(tc.tile_pool(name="psd", bufs=2, space="PSUM"))
        ps_hm = ctx.enter_context(tc.tile_pool(name="psh", bufs=4, space="PSUM"))
        ps_o = ctx.enter_context(tc.tile_pool(name="pso", bufs=1, space="PSUM"))

        vpn_sb = consts.tile([128, 5, 320], BF16)
        wct_sb = consts.tile([128, 5, 128], BF16)
        phi_sb = consts.tile([128, BL * T], BF16)
        st_sb = consts.tile([128, 3, 128], BF16)  # S_T chunks [(jfi,b), c, rc]
        # rows 32r+jf (r<4): S[jf] replicated; rows 6+25jf: staging
        s_rep = consts.tile([128, BL * 128], BF16)
        u_sb = consts.tile([128, T, 7], BF16)
        ft_sb = consts.tile([128, 5, BL], BF16)
        wx_sb = consts.tile([128, 5, 7], BF16)
        bx_sb = consts.tile([7, 1], F32)
        hm_sb = hmp.tile([128, BL * T], BF16)
        tiny = consts.tile([1, 1], F32)

        # bulk inputs: one big dma_start each
        nc.scalar.dma_start(out=wct_sb[:], in_=wct_d[:])
        nc.sync.dma_start(out=vpn_sb[:], in_=vpn_d[:])
        for rr in range(R):
            eng = nc.scalar if rr % 2 == 0 else nc.sync
            eng.dma_start(out=phi_sb[32 * rr:32 * rr + 5, :],
                          in_=phi_d[rr, :, :])
        nc.sync.dma_start(out=ft_sb[:], in_=ft_d[:])
        nc.gpsimd.dma_start(out=u_sb[:], in_=u_d[:])
        nc.gpsimd.dma_start(out=wx_sb[:], in_=wx_d[:])
        nc.gpsimd.dma_start(out=bx_sb[:], in_=bx_d[:])

        # preload ACT's table set (has Relu); issued after the DMA starts
        # so it doesn't block the scalar ring
        nc.vector.memset(tiny[:], 0.0)
        nc.scalar.activation(out=tiny[:], in_=tiny[:],
                             func=mybir.ActivationFunctionType.Relu)
        # warm-up matmuls: keep the PE array busy through the input-DMA
        # window so HAM unthrottles (1.2 -> 2.4 GHz) before stage D; a
        # [128,1] lhsT makes LDWEIGHTS ~free (cost scales with columns)
        dum_w = consts.tile([128, 1], BF16)
        dum_m = consts.tile([128, 512], BF16)
        nc.vector.memset(dum_w[:], 1.0)
        nc.vector.memset(dum_m[:], 1.0)
        ps_w = ctx.enter_context(tc.tile_pool(name="psw", bufs=1, space="PSUM"))
        dum_p = ps_w.tile([1, 512], F32, padded_shape=[None, 512])
        for _ in range(4):
            nc.tensor.matmul(out=dum_p[:], lhsT=dum_w[:], rhs=dum_m[:],
                             start=True, stop=True, skip_group_check=True)

        hm3 = hm_sb.rearrange("p (b t) -> p b t", t=T)

        # ---- stage D: S_T[(jfi, b), (rc)] = vpn.T @ wct, M-chunks ----
        # chunk c covers vpn cols 128c..128c+mp = jf {2c, 2c+1} (c<2), {4}
        for c in range(3):
            mp = 128 if c < 2 else 64
            pt = ps_d.tile([128, 128], F32, tag="sd", padded_shape=[None, 512])
            for kt in range(5):
                kp = KPD[kt]
                nc.tensor.matmul(out=pt[0:mp, :],
                                 lhsT=vpn_sb[0:kp, kt, 128 * c:128 * c + mp],
                                 rhs=wct_sb[0:kp, kt, :],
                                 start=(kt == 0), stop=(kt == 4))
            nc.vector.tensor_copy(st_sb[0:mp, c, :], pt[0:mp, :])

        # keep the PE warm through the realign/expand DMA window; the
        # st_sb read makes these depend on the last stage-D copy so the
        # scheduler cannot hoist them ahead of it (which would delay the
        # realign chain)
        for _ in range(48):
            nc.tensor.matmul(out=dum_p[:, 0:128], lhsT=dum_w[:],
                             rhs=st_sb[:, 2, :],
                             start=True, stop=True, skip_group_check=True)

        # ---- realign: st_sb -> staging rows {21+4jf} of s_rep ----
        # (single-dest-partition DMAs; the 5 dests hit 5 distinct SDMA
        # engines so the transfers run in parallel)
        engs3 = [nc.sync, nc.scalar, nc.gpsimd]
        for jf in range(JF):
            jfi, c = jf % 2, jf // 2
            row = 6 + 25 * jf
            engs3[jf % 3].dma_start(
                out=s_rep[row:row + 1, :],
                in_=st_sb[64 * jfi:64 * jfi + 64, c, :])

        # ---- replicate: staging -> rows 5k+jf for k=0..3 (bulk copies,
        # contiguous partition-dim0 both sides, 16KB descriptors) ----
        srb = s_rep[:]
        for k in range(R):
            out_k = AP(srb.tensor, (32 * k) * 8192, [[8192, 5], [1, 8192]])
            in_k = AP(srb.tensor, 6 * 8192, [[25 * 8192, 5], [1, 8192]])
            engs3[k % 3].dma_start(out=out_k, in_=in_k)

        if dbg:
            nc.sync.dma_start(out=dsb_d[:], in_=s_rep[:])

        # ---- stage A: Hm[(rc), t] per b: one K=20, M=128, N=150 MM ----
        state = {"flip": 0}

        def stage_a(bs):
            """Hm for a chunk of 3 consecutive batches (one PSUM bank)."""
            pt = ps_hm.tile([128, 450], F32, tag="hmps",
                            padded_shape=[None, 512])
            for slot, b in enumerate(bs):
                for r in range(R):
                    nc.tensor.matmul(
                        out=pt[32 * r:32 * r + 32,
                               150 * slot:150 * slot + 150],
                        lhsT=s_rep[32 * r:32 * r + 5,
                                   128 * b + 32 * r:128 * b + 32 * r + 32],
                        rhs=phi_sb[32 * r:32 * r + 5, 150 * b:150 * b + 150],
                        start=True, stop=True,
                        tile_position=(32 * r, 32 * r),
                        skip_group_check=True,
                    )
            o = hm3[:, bs[0]:bs[-1] + 1, :]
            state["flip"] += 1
            if state["flip"] % 2:
                nc.scalar.activation(
                    out=o, in_=pt[:, 0:150 * len(bs)],
                    func=mybir.ActivationFunctionType.Relu)
            else:
                nc.vector.tensor_scalar_max(
                    out=o, in0=pt[:, 0:150 * len(bs)], scalar1=0.0)

        for i in range(0, BL, 3):
            stage_a(list(range(i, min(i + 3, BL))))

        # ---- final: out[i,b] = sum_{(rc),t} relu(Hm)*U + sum f*Wx + bx
        op = ps_o.tile([128, BL], F32, padded_shape=[None, 512])
        glast = {gg: max(t for t in range(T) if t % 4 == gg) for gg in range(4)}
        for t in range(T):
            g = t % 4
            nc.tensor.matmul(
                out=op[32 * g:32 * g + 7, :],
                lhsT=u_sb[:, t, :],
                rhs=hm3[:, :, t],
                start=(t < 4),
                stop=(g != 0 and t == glast[g]),
                tile_position=(0, 32 * g),
                skip_group_check=True,
            )
        for kt, (k0, kp) in enumerate(KTS):
            nc.tensor.matmul(
                out=op[0:7, :],
                lhsT=wx_sb[0:kp, kt, :],
                rhs=ft_sb[0:kp, kt, :],
                start=False, stop=(kt == 4),
                tile_position=(0, 0),
                skip_group_check=True,
            )

        # tail: sum the 4 col groups + bias
        c1 = consts.tile([7, BL], F32)
        c2 = consts.tile([7, BL], F32)
        s1 = consts.tile([7, BL], F32)
        s2 = consts.tile([7, BL], F32)
        ob = consts.tile([7, BL], F32)
        nc.vector.tensor_copy(c1[:], op[32:39, :])
        nc.scalar.copy(c2[:], op[96:103, :])
        nc.vector.scalar_tensor_tensor(
            out=s1[:], in0=op[0:7, :], scalar=bx_sb[:], in1=c1[:],
            op0=mybir.AluOpType.add, op1=mybir.AluOpType.add)
        nc.vector.scalar_tensor_tensor(
            out=s2[:], in0=op[64:71, :], scalar=0.0, in1=c2[:],
            op0=mybir.AluOpType.add, op1=mybir.AluOpType.add)
        nc.vector.tensor_add(ob[:], s1[:], s2[:])
        nc.sync.dma_start(out=out_d[:], in_=ob[:])

    nc.finalize()
    return nc


def _fit_poly(u, v):
    xmax = float(np.abs(u).max()) * float(np.abs(v).max()) * 1.02 + 1e-30
    xs = xmax * np.sin(np.linspace(-np.pi / 2, np.pi / 2, 4001))
    A = xs[:, None] ** np.array(POWS)[None, :]
    w = 1.0 / (0.05 + np.abs(xs))
    coef, *_ = np.linalg.lstsq(A * w[:, None], np.tanh(xs) * w, rcond=None)
    return coef


def _host_prep(feats, a, W, Wc, Wh, W1, b1, W2, b2):
    f = feats[:, :, 0, :]                              # [R,B,T]
    u = a[:, None, None] * f                           # [R,B,T]
    v = feats.reshape(B, K)                            # [B,K]
    coef = _fit_poly(u, v)
    Wx = W2 @ W1                                       # [7,K]
    bx = W2 @ b1 + b2                                  # [7]

    # U[(rc), t, i] = Wh[r,c] * Wx[i, r*T+t]
    U = np.zeros((128, T, 7), np.float32)
    for r in range(R):
        blk = Wx[:, r * T:(r + 1) * T].T               # [T,7]
        U[r * H:(r + 1) * H] = Wh[r][:, None, None] * blk[None]

    # wct[k, kt, 32r+c] = Wc[r, c, k0+k]; row 88 of kt4 = W/a (f*W fold)
    wct = np.zeros((128, 5, 128), np.float32)
    for kt, (k0, kp) in enumerate(KTS):
        for r in range(R):
            wct[:kp, kt, 32 * r:32 * (r + 1)] = Wc[r, :, k0:k0 + kp].T
    wct[88, 4, :] = (W / a[:, None]).reshape(128)

    wx_h = np.zeros((128, 5, 7), np.float32)
    for kt, (k0, kp) in enumerate(KTS):
        wx_h[:kp, kt, :] = Wx[:, k0:k0 + kp].T

    fT_full = np.concatenate([f[r].T for r in range(R)], axis=0)  # [K, B]

    # basis powers with the tanh-poly coefficients folded into the v side
    vbasis = np.stack([coef[j] * v ** POWS[j] for j in range(JF)], 0)  # [J,B,K]
    ubasis = np.stack([u ** POWS[j] for j in range(JF)], 0)            # [J,R,B,T]

    in_maps = []
    for mcore in range(NCORES):
        b0 = mcore * BL
        # phi[r, jf, 150*b + t] = u[r, b0+b, t]^p_jf
        phi = np.zeros((4, 5, BL * T), np.float32)
        for r in range(R):
            for jf in range(JF):
                phi[r, jf] = ubasis[jf, r, b0:b0 + BL, :].reshape(-1)
        # vpn[k, kt, 64*jf + b] = c_jf v[b0+b, k0+k]^p_jf
        vpn = np.zeros((128, 5, 320), np.float32)
        for kt, (k0, kp) in enumerate(KTS):
            vb = vbasis[:, b0:b0 + BL, k0:k0 + kp]     # [J, BL, kp]
            vpn[:kp, kt, :] = vb.reshape(JF * BL, kp).T
        vpn[88, 4, 0:64] = 1.0                         # jf=0 block indicator
        ft_h = np.zeros((128, 5, BL), np.float32)
        for kt, (k0, kp) in enumerate(KTS):
            ft_h[:kp, kt, :] = fT_full[k0:k0 + kp, b0:b0 + BL]
        in_maps.append({
            "phi": phi.astype(BF),
            "vpn": vpn.astype(BF),
            "wct": wct.astype(BF),
            "u": U.astype(BF),
            "ft": ft_h.astype(BF),
            "wx": wx_h.astype(BF),
            "bx": bx.astype(np.float32).reshape(7, 1),
        })
    return in_maps


def kernel(feats_list, a, W, Wc, Wh, W1, b1, W2, b2):
    feats = np.asarray(feats_list, np.float32)
    in_maps = _host_prep(
        feats,
        np.asarray(a, np.float32),
        np.asarray(W, np.float32),
        np.asarray(Wc, np.float32),
        np.asarray(Wh, np.float32),
        np.asarray(W1, np.float32),
        np.asarray(b1, np.float32),
        np.asarray(W2, np.float32),
        np.asarray(b2, np.float32),
    )
    if "nc" not in _CACHE:
        _CACHE["nc"] = build_nc()
    res = bass_utils.run_bass_kernel_spmd(
        _CACHE["nc"], in_maps, core_ids=list(range(NCORES))
    )
    _CACHE["last_result"] = res
    out = np.concatenate([r["out"].T for r in res.results], axis=0)  # [B,7]
    return out[:, None, :].astype(np.float32)                        # [B,1,7]


# revision 14
# speedup vs baseline: 1.2566x; 1.2566x over previous
"""Trainium2 Bass kernel for nn_CAM_62852551409742 (low-rank tanh rewrite, v2).

Math (reference):
  f = feats[:, :, 0, :]                               [R,B,T], R=4, B=512, T=150
  v = feats.reshape(B, K)                             [B,K], K=600
  att[r,b,t,k] = tanh(u[r,b,t] * v[b,k]),  u = a[r]*f
  Hm = relu(att @ Wc[r].T + f*W[r])                   [R,B,T,32]
  attf = Hm @ Wh[r] + f
  out = (attf-cat @ W1.T + b1) @ W2.T + b2            [B,1,7]

Key rewrite (kept from v1): tanh(uv) ~= sum_j c_j (uv)^p_j (odd powers
p=1,3,5,7,9), so att @ Wc.T = sum_j u^p_j * S_j with
S_j[r,b,c] = sum_k c_j v[b,k]^p_j Wc[r,c,k]; the f*W term folds in via an
indicator row (row 88 of the kt=4 contraction tile).

v2 pipeline (v1 profile: ~12us stall -- phi lived on 5 SBUF partitions
~= 2 of 16 SDMA ports; 28 dma_starts at ~0.7us sequencer each; 256
stage-A MMs; cold PE for the back half):

  stage D: S_T[(jfi,b), (rc)] = vpn.T @ wct as 3 M-chunks x 5 kt
           accumulating MMs (15 total; S comes off the PE already
           transposed -- no PE-transposes, no identity matrix).
  realign: 3 DVE copies -> st_sb, then 5 single-dest-partition DMAs
           -> s_all[jf, 128 b + (rc)]  (DMA partition steps are only
           legal on dim0 of BOTH access patterns, so a partition-axis
           exchange needs a single-partition side).
  expand:  4 DMAs (one per r, both sides partition-step on dim0)
           -> s_blk[5r + jf, 128 b + 32 r + c], block-diagonal over r
           with zeros from one early DVE memset.
  stage A: ONE plain matmul per batch: K=20 (5r+jf rows), M=128, N=150.
           64 MMs (was 256).  phi20 [20, 9600] (partition 5r+jf) is a
           single ~2.9us DMA over 5 SDMA engines (was ~7us over 2).
  relu:    PSUM->SBUF bf16 drain per 3 batches, DVE/ACT alternating
           (the ~6us floor: 1.23M fp32 PSUM reads at 1x).
  final:   per-t MMs vs U[(rc),t,i]=Wh*Wx in 4 col groups + f@Wx tail
           (unchanged from v1).

DMA budget (each dma_start ~0.7us of its ring's sequencer): sync ring:
vpn, realign x5, expand r0/r1, out; scalar ring: wct, phi, ft, expand
r2/r3; gpsimd (SWDGE): u, wx, bx.
"""

import os
from contextlib import ExitStack

import numpy as np
import ml_dtypes

import concourse.bacc as bacc
import concourse.bass as bass
import concourse.tile as tile
from concourse import mybir
from concourse import bass_utils
from concourse.ap import AP

R, B, T, H = 4, 512, 150, 32
K = R * T                      # 600
NCORES = 8
BL = B // NCORES               # 64 batches per core
JF = 5                         # odd powers 1..9
POWS = (1, 3, 5, 7, 9)
KTS = [(0, 128), (128, 128), (256, 128), (384, 128), (512, 88)]
KPD = [128, 128, 128, 128, 89]   # stage-D contraction (kt4 + indicator row)
F32 = mybir.dt.float32
BF16 = mybir.dt.bfloat16
BF = ml_dtypes.bfloat16

_CACHE = {}


def build_nc():
    nc = bacc.Bacc("TRN2", target_bir_lowering=False)
    # vpn cols: (jf, b) -> 64*jf + b
    vpn_d = nc.dram_tensor("vpn", [128, 5, 320], BF16, kind="ExternalInput")
    wct_d = nc.dram_tensor("wct", [128, 5, 128], BF16, kind="ExternalInput")
    # phi20[5r + jf, 150*b + t] = u[r, b0+b, t]^p_jf
    phi_d = nc.dram_tensor("phi", [20, BL * T], BF16, kind="ExternalInput")
    u_d = nc.dram_tensor("u", [128, T, 7], BF16, kind="ExternalInput")
    ft_d = nc.dram_tensor("ft", [128, 5, BL], BF16, kind="ExternalInput")
    wx_d = nc.dram_tensor("wx", [128, 5, 7], BF16, kind="ExternalInput")
    bx_d = nc.dram_tensor("bx", [7, 1], F32, kind="ExternalInput")
    out_d = nc.dram_tensor("out", [7, BL], F32, kind="ExternalOutput")
    dbg = os.environ.get("KDEBUG")
    if dbg:
        dsb_d = nc.dram_tensor("dsb", [20, 8192], BF16, kind="ExternalOutput")

    with tile.TileContext(nc) as tc, ExitStack() as ctx:
        consts = ctx.enter_context(tc.tile_pool(name="consts", bufs=1))
        hmp = ctx.enter_context(tc.tile_pool(name="hm", bufs=1))
        ps_d = ctx.enter_context(tc.tile_pool(name="psd", bufs=2, space="PSUM"))
        ps_hm = ctx.enter_context(tc.tile_pool(name="psh", bufs=4, space="PSUM"))
        ps_o = ctx.enter_context(tc.tile_pool(name="pso", bufs=1, space="PSUM"))

        vpn_sb = consts.tile([128, 5, 320], BF16)
        wct_sb = consts.tile([128, 5, 128], BF16)
        phi_sb = consts.tile([20, BL * T], BF16)
        st_sb = consts.tile([128, 3, 128], BF16)  # S_T chunks [(jfi,b), c, rc]
        s_all = consts.tile([17, BL * 128], BF16)  # [4*jf, 128 b + (rc)]
        s_blk = consts.tile([20, BL * 128], BF16)  # [5r+jf, 128 b + 32 r + c]
        u_sb = consts.tile([128, T, 7], BF16)
        ft_sb = consts.tile([128, 5, BL], BF16)
        wx_sb = consts.tile([128, 5, 7], BF16)
        bx_sb = consts.tile([7, 1], F32)
        hm_sb = hmp.tile([128, BL * T], BF16)
        tiny = consts.tile([1, 1], F32)

        # bulk inputs: one big dma_start each
        nc.scalar.dma_start(out=wct_sb[:], in_=wct_d[:])
        nc.sync.dma_start(out=vpn_sb[:], in_=vpn_d[:])
        nc.scalar.dma_start(out=phi_sb[:], in_=phi_d[:])
        nc.sync.dma_start(out=ft_sb[:], in_=ft_d[:])
        nc.sync.dma_start(out=u_sb[:], in_=u_d[:])
        nc.gpsimd.dma_start(out=wx_sb[:], in_=wx_d[:])
        nc.gpsimd.dma_start(out=bx_sb[:], in_=bx_d[:])

        # preload ACT's table set (has Relu); issued after the DMA starts
        # so it doesn't block the scalar ring
        nc.vector.memset(tiny[:], 0.0)
        nc.scalar.activation(out=tiny[:], in_=tiny[:],
                             func=mybir.ActivationFunctionType.Relu)
        # warm-up matmuls: keep the PE array busy through the input-DMA
        # window so HAM unthrottles (1.2 -> 2.4 GHz) before stage D; a
        # [128,1] lhsT makes LDWEIGHTS ~free (cost scales with columns)
        dum_w = consts.tile([128, 1], BF16)
        dum_m = consts.tile([128, 512], BF16)
        nc.vector.memset(dum_w[:], 1.0)
        nc.vector.memset(dum_m[:], 1.0)
        ps_w = ctx.enter_context(tc.tile_pool(name="psw", bufs=1, space="PSUM"))
        dum_p = ps_w.tile([1, 512], F32, padded_shape=[None, 512])
        for _ in range(4):
            nc.tensor.matmul(out=dum_p[:], lhsT=dum_w[:], rhs=dum_m[:],
                             start=True, stop=True, skip_group_check=True)

        hm3 = hm_sb.rearrange("p (b t) -> p b t", t=T)

        # ---- stage D: S_T[(jfi, b), (rc)] = vpn.T @ wct, M-chunks ----
        # chunk c covers vpn cols 128c..128c+mp = jf {2c, 2c+1} (c<2), {4}
        for c in range(3):
            mp = 128 if c < 2 else 64
            pt = ps_d.tile([128, 128], F32, tag="sd", padded_shape=[None, 512])
            for kt in range(5):
                kp = KPD[kt]
                nc.tensor.matmul(out=pt[0:mp, :],
                                 lhsT=vpn_sb[0:kp, kt, 128 * c:128 * c + mp],
                                 rhs=wct_sb[0:kp, kt, :],
                                 start=(kt == 0), stop=(kt == 4))
            nc.vector.tensor_copy(st_sb[0:mp, c, :], pt[0:mp, :])

        # zero the block-diag lhsT before the expands land (split across
        # DVE and GpSimd so neither blocks the stage-D PSUM copies)
        nc.vector.memset(s_blk[:, 0:4096], 0.0)
        nc.gpsimd.memset(s_blk[:, 4096:8192], 0.0)
        # keep the PE warm through the realign/expand DMA window; the
        # st_sb read makes these depend on the last stage-D copy so the
        # scheduler cannot hoist them ahead of it (which would delay the
        # realign chain)
        for _ in range(48):
            nc.tensor.matmul(out=dum_p[:, 0:128], lhsT=dum_w[:],
                             rhs=st_sb[:, 2, :],
                             start=True, stop=True, skip_group_check=True)

        # ---- realign: st_sb -> s_all rows {4jf} (5 distinct SDMA
        # engines), spread over all three DGE rings ----
        engs3 = [nc.sync, nc.scalar, nc.gpsimd]
        for jf in range(JF):
            jfi, c = jf % 2, jf // 2
            engs3[jf % 3].dma_start(
                out=s_all[4 * jf:4 * jf + 1, :],
                in_=st_sb[64 * jfi:64 * jfi + 64, c, :])

        # ---- expand: s_all -> s_blk block-diagonal, 1 DMA per r ----
        sa4 = s_all[0:17:4, :].rearrange("j (b rc) -> j b rc", rc=128)
        sbb = s_blk[:]
        for r in range(R):
            out_r = AP(sbb.tensor, (5 * r) * 8192 + 32 * r,
                       [[8192, 5], [128, 64], [1, 32]])
            engs3[r % 3].dma_start(out=out_r, in_=sa4[:, :, 32 * r:32 * r + 32])

        if dbg:
            nc.sync.dma_start(out=dsb_d[:], in_=s_blk[:])

        # ---- stage A: Hm[(rc), t] per b: one K=20, M=128, N=150 MM ----
        state = {"flip": 0}

        def stage_a(bs):
            """Hm for a chunk of 3 consecutive batches (one PSUM bank)."""
            pt = ps_hm.tile([128, 450], F32, tag="hmps",
                            padded_shape=[None, 512])
            for slot, b in enumerate(bs):
                nc.tensor.matmul(
                    out=pt[:, 150 * slot:150 * slot + 150],
                    lhsT=s_blk[0:20, 128 * b:128 * b + 128],
                    rhs=phi_sb[0:20, 150 * b:150 * b + 150],
                    start=True, stop=True,
                    skip_group_check=True,
                )
            o = hm3[:, bs[0]:bs[-1] + 1, :]
            state["flip"] += 1
            if state["flip"] % 2:
                nc.scalar.activation(
                    out=o, in_=pt[:, 0:150 * len(bs)],
                    func=mybir.ActivationFunctionType.Relu)
            else:
                nc.vector.tensor_scalar_max(
                    out=o, in0=pt[:, 0:150 * len(bs)], scalar1=0.0)

        for i in range(0, BL, 3):
            stage_a(list(range(i, min(i + 3, BL))))

        # ---- final: out[i,b] = sum_{(rc),t} relu(Hm)*U + sum f*Wx + bx
        op = ps_o.tile([128, BL], F32, padded_shape=[None, 512])
        glast = {gg: max(t for t in range(T) if t % 4 == gg) for gg in range(4)}
        for t in range(T):
            g = t % 4
            nc.tensor.matmul(
                out=op[32 * g:32 * g + 7, :],
                lhsT=u_sb[:, t, :],
                rhs=hm3[:, :, t],
                start=(t < 4),
                stop=(g != 0 and t == glast[g]),
                tile_position=(0, 32 * g),
                skip_group_check=True,
            )
        for kt, (k0, kp) in enumerate(KTS):
            nc.tensor.matmul(
                out=op[0:7, :],
                lhsT=wx_sb[0:kp, kt, :],
                rhs=ft_sb[0:kp, kt, :],
                start=False, stop=(kt == 4),
                tile_position=(0, 0),
                skip_group_check=True,
            )

        # tail: sum the 4 col groups + bias
        c1 = consts.tile([7, BL], F32)
        c2 = consts.tile([7, BL], F32)
        s1 = consts.tile([7, BL], F32)
        s2 = consts.tile([7, BL], F32)
        ob = consts.tile([7, BL], F32)
        nc.vector.tensor_copy(c1[:], op[32:39, :])
        nc.scalar.copy(c2[:], op[96:103, :])
        nc.vector.scalar_tensor_tensor(
            out=s1[:], in0=op[0:7, :], scalar=bx_sb[:], in1=c1[:],
            op0=mybir.AluOpType.add, op1=mybir.AluOpType.add)
        nc.vector.scalar_tensor_tensor(
            out=s2[:], in0=op[64:71, :], scalar=0.0, in1=c2[:],
            op0=mybir.AluOpType.add, op1=mybir.AluOpType.add)
        nc.vector.tensor_add(ob[:], s1[:], s2[:])
        nc.sync.dma_start(out=out_d[:], in_=ob[:])

    nc.finalize()
    return nc


def _fit_poly(u, v):
    xmax = float(np.abs(u).max()) * float(np.abs(v).max()) * 1.02 + 1e-30
    xs = xmax * np.sin(np.linspace(-np.pi / 2, np.pi / 2, 4001))
    A = xs[:, None] ** np.array(POWS)[None, :]
    w = 1.0 / (0.05 + np.abs(xs))
    coef, *_ = np.linalg.lstsq(A * w[:, None], np.tanh(xs) * w, rcond=None)
    return coef


def _host_prep(feats, a, W, Wc, Wh, W1, b1, W2, b2):
    f = feats[:, :, 0, :]                              # [R,B,T]
    u = a[:, None, None] * f                           # [R,B,T]
    v = feats.reshape(B, K)                            # [B,K]
    coef = _fit_poly(u, v)
    Wx = W2 @ W1                                       # [7,K]
    bx = W2 @ b1 + b2                                  # [7]

    # U[(rc), t, i] = Wh[r,c] * Wx[i, r*T+t]
    U = np.zeros((128, T, 7), np.float32)
    for r in range(R):
        blk = Wx[:, r * T:(r + 1) * T].T               # [T,7]
        U[r * H:(r + 1) * H] = Wh[r][:, None, None] * blk[None]

    # wct[k, kt, 32r+c] = Wc[r, c, k0+k]; row 88 of kt4 = W/a (f*W fold)
    wct = np.zeros((128, 5, 128), np.float32)
    for kt, (k0, kp) in enumerate(KTS):
        for r in range(R):
            wct[:kp, kt, 32 * r:32 * (r + 1)] = Wc[r, :, k0:k0 + kp].T
    wct[88, 4, :] = (W / a[:, None]).reshape(128)

    wx_h = np.zeros((128, 5, 7), np.float32)
    for kt, (k0, kp) in enumerate(KTS):
        wx_h[:kp, kt, :] = Wx[:, k0:k0 + kp].T

    fT_full = np.concatenate([f[r].T for r in range(R)], axis=0)  # [K, B]

    # basis powers with the tanh-poly coefficients folded into the v side
    vbasis = np.stack([coef[j] * v ** POWS[j] for j in range(JF)], 0)  # [J,B,K]
    ubasis = np.stack([u ** POWS[j] for j in range(JF)], 0)            # [J,R,B,T]

    in_maps = []
    for mcore in range(NCORES):
        b0 = mcore * BL
        # phi20[5r + jf, 150*b + t] = u[r, b0+b, t]^p_jf
        phi = np.zeros((20, BL * T), np.float32)
        for r in range(R):
            for jf in range(JF):
                phi[5 * r + jf] = ubasis[jf, r, b0:b0 + BL, :].reshape(-1)
        # vpn[k, kt, 64*jf + b] = c_jf v[b0+b, k0+k]^p_jf
        vpn = np.zeros((128, 5, 320), np.float32)
        for kt, (k0, kp) in enumerate(KTS):
            vb = vbasis[:, b0:b0 + BL, k0:k0 + kp]     # [J, BL, kp]
            vpn[:kp, kt, :] = vb.reshape(JF * BL, kp).T
        vpn[88, 4, 0:64] = 1.0                         # jf=0 block indicator
        ft_h = np.zeros((128, 5, BL), np.float32)
        for kt, (k0, kp) in enumerate(KTS):
            ft_h[:kp, kt, :] = fT_full[k0:k0 + kp, b0:b0 + BL]
        in_maps.append({
            "phi": phi.astype(BF),
            "vpn": vpn.astype(BF),
            "wct": wct.astype(BF),
            "u": U.astype(BF),
            "ft": ft_h.astype(BF),
            "wx": wx_h.astype(BF),
            "bx": bx.astype(np.float32).reshape(7, 1),
        })
    return in_maps


def kernel(feats_list, a, W, Wc, Wh, W1, b1, W2, b2):
    feats = np.asarray(feats_list, np.float32)
    in_maps = _host_prep(
        feats,
        np.asarray(a, np.float32),
        np.asarray(W, np.float32),
        np.asarray(Wc, np.float32),
        np.asarray(Wh, np.float32),
        np.asarray(W1, np.float32),
        np.asarray(b1, np.float32),
        np.asarray(W2, np.float32),
        np.asarray(b2, np.float32),
    )
    if "nc" not in _CACHE:
        _CACHE["nc"] = build_nc()
    res = bass_utils.run_bass_kernel_spmd(
        _CACHE["nc"], in_maps, core_ids=list(range(NCORES))
    )
    _CACHE["last_result"] = res
    out = np.concatenate([r["out"].T for r in res.results], axis=0)  # [B,7]
    return out[:, None, :].astype(np.float32)                        # [B,1,7]
